# revision 20
# baseline (speedup 1.0000x reference)
"""Trainium2 Bass kernel for CausalSelfAttention (RoPE + GQA), 8-core SPMD.

Sharding: 8 cores = 4 batches x 2 query-halves (as v1). Keys PERMUTED per
core so slot s consumes the static key-chunk range [2s, 2s+PAD_s); the first
1024 permuted keys ARE the core's queries, so Q projection reuses the same
x input and RoPE tables.

v2 changes vs v1 (236us):
  - All projections in bf16 (PE cost is free-dim rows only; bf16 allows
    free<256 at full rate and halves DMA). RoPE still via double projection
    (normal + pair-swapped weights), but packed into single weight matrices
    (Q+Qs = 1152 = 9x128 cols, K+Ks = 384 = 3x128) so no 64-row matmuls.
  - PV transposed: out[q,65] = P_chunk^T @ V[keys,65] with bf16 V moving
    (free 65 vs 256 -> PV PE cost halved); ones-column gives the softmax
    denominator per q-PARTITION, so the divide is a per-partition scalar op
    (gpsimd normalize_recip) instead of reciprocal+partition_broadcast+mul.
  - y [q, feat] transposed back for the output projection with PE bf16
    transposes via an identity matrix (cheap: 128 rows each).
  - exp -> bf16 P; mask multiply all-bf16 on DVE (2x mode).
  - Projection/transpose/oproj work is emitted through a filler queue
    interleaved between attention heads so PE fills the gaps of the
    ACT(exp)-paced attention stream.
"""
import sys

sys.path.insert(0, "/opt/trn_rl_repo")

import numpy as np
import ml_dtypes

B, T, C = 4, 2048, 576
H, HKV, D = 9, 3, 64
THETA = 10000.0
QB = 256                      # query block
TQ = 1024                     # queries per core
SLOT_PAD = [16, 12, 8, 4]     # padded key-chunk counts per slot
QBLOCKS = [[7, 5, 2, 0], [6, 4, 3, 1]]   # q-256-block ids per half j
KEYORDER = [[7, 5, 2, 0, 1, 3, 4, 6], [6, 4, 3, 1, 0, 2, 5, 7]]
CCX = [(0, 128), (128, 128), (256, 128), (384, 128), (512, 65)]   # 577 rows incl ones
CCQ = [(0, 128), (128, 128), (256, 128), (384, 128), (512, 64)]   # 576-row chunks
MM = [(0, 128), (128, 128), (256, 128), (384, 128), (512, 64)]    # 576 out chunks


def _slot_seq(s):
    """Key-chunk emission order for slot s: fulls, then the two diag chunks."""
    return list(range(2 * s + 2, 2 * s + SLOT_PAD[s])) + [2 * s, 2 * s + 1]


_PROG = None


def _build_program():
    import concourse.bacc as bacc
    import concourse.mybir as mybir
    import concourse.tile as tile

    dt = mybir.dt
    f32, bf16, fp8 = dt.float32, dt.bfloat16, dt.float8e4
    AF = mybir.ActivationFunctionType

    nc = bacc.Bacc("TRN2", target_bir_lowering=False, debug=False, num_devices=8)

    def inp(name, shape, d):
        return nc.declare_dram_parameter(name, shape, d, isOutput=False)

    xkT = inp("xkT", [577, T], bf16)
    wqqs = inp("wqqs", [C, 2 * C], bf16)
    wkks = inp("wkks", [C, 2 * HKV * D], bf16)
    wvp = inp("wvp", [577, 195], bf16)
    woT = inp("woT", [C, C], bf16)
    c2k = inp("c2k", [128, T], bf16)
    s2k = inp("s2k", [128, T], bf16)
    masksp = inp("masks", [128, 16 * QB], bf16)
    idenp = inp("iden", [128, 128], bf16)
    # output in partition-major layout [128, 5, TQ]: row f=128m+p of the
    # logical [576, TQ] lives at [p, m, :]; host reassembles.
    yT = nc.declare_dram_parameter("yT", [128, 5, TQ], f32, isOutput=True)

    with tile.TileContext(nc) as tc:
        with (
            tc.tile_pool(name="const", bufs=1) as cp,
            tc.tile_pool(name="rope", bufs=2) as rp,
            tc.tile_pool(name="pwork", bufs=3) as pw,
            tc.tile_pool(name="ysb", bufs=2) as ysbp,
            tc.tile_pool(name="ost", bufs=2) as ostp,
            tc.tile_pool(name="psS", bufs=2, space="PSUM") as psS,
            tc.tile_pool(name="psY", bufs=2, space="PSUM") as psY,
        ):
            # ---------------- persistent constants ----------------
            # (DMA emission for most constants is deferred into the
            # projection phase so the first window's x/wk loads go first.)
            wo_r = [cp.tile([128, C], bf16, tag=f"wo{i}", name=f"wo{i}")
                    for i in range(5)]
            m_b = cp.tile([128, 16 * QB], bf16, tag="masks", name="masks")
            c2k_t = cp.tile([128, T], bf16, tag="c2k", name="c2k")
            s2k_t = cp.tile([128, T], bf16, tag="s2k", name="s2k")
            id_t = cp.tile([128, 128], bf16, tag="iden", name="iden")

            kt_h = [cp.tile([64, T], bf16, tag=f"kt{g}", name=f"kt{g}")
                    for g in range(HKV)]
            qth = [cp.tile([64, TQ], bf16, tag=f"qth{h}", name=f"qth{h}")
                   for h in range(H)]
            v_t = [cp.tile([128, 195], bf16, tag=f"v{c}", name=f"v{c}")
                   for c in range(16)]
            yq = [cp.tile([128, C], bf16, tag=f"yq{q}", name=f"yq{q}")
                  for q in range(8)]
            ypr = [cp.tile([128, TQ], bf16, tag=f"ypr{p}", name=f"ypr{p}")
                   for p in range(5)]

            # ---------------- attention ----------------
            fillers = []
            qdone = {0: set(), 1: set()}
            kdone = set()
            vdone = set()

            def pump(k=1):
                for _ in range(k):
                    if fillers:
                        fillers.pop(0)()

            def attn_slot(s, after_head=None, pump_k=1, pump_plan=None):
                seq = _slot_seq(s)
                n = len(seq)
                site = [0]
                win = 1 if s >= 2 else 0
                for h in range(H):
                    g = h // 3
                    yh = [psY.tile([128, 65], f32, tag="ypsum", name="ypsum",
                                   padded_shape=[128, 512]) for _ in range(2)]
                    for sc in range(n // 4):
                        grp = seq[4 * sc:4 * sc + 4]
                        while (h not in qdone[win]
                               or any(c not in vdone for c in grp)
                               or any((c // 4) not in kdone for c in grp)):
                            assert fillers, (
                                f"slot {s} head {h} group {sc} deps not in "
                                f"filler queue")
                            fillers.pop(0)()
                        sp = psS.tile([128, 4 * QB], f32, tag="scores",
                                      name="scores")
                        for i in range(4):
                            c = seq[4 * sc + i]
                            nc.tensor.matmul(
                                sp[:, QB * i:QB * (i + 1)],
                                kt_h[g][0:64, 128 * c:128 * (c + 1)],
                                qth[h][0:64, QB * s:QB * (s + 1)],
                                start=True, stop=True)
                        p_b = pw.tile([128, 4 * QB], bf16, tag="p", name="p")
                        nc.scalar.activation(p_b[:], sp[:], AF.Exp, scale=0.125)
                        if sc == n // 4 - 1:
                            nc.vector.tensor_mul(
                                p_b[:], p_b[:],
                                m_b[:, 1024 * s:1024 * (s + 1)])
                        for i in range(4):
                            c = seq[4 * sc + i]
                            ci = 4 * sc + i
                            for hf in range(2):
                                nc.tensor.matmul(
                                    yh[hf][:, 0:65],
                                    p_b[:, QB * i + 128 * hf:
                                        QB * i + 128 * hf + 128],
                                    v_t[c][:, 65 * g:65 * g + 65],
                                    start=(ci == 0), stop=(ci == n - 1))
                        if pump_plan is not None:
                            k = (pump_plan[site[0]]
                                 if site[0] < len(pump_plan) else 1)
                            site[0] += 1
                            pump(k)
                        else:
                            pump(pump_k)
                    for hf in range(2):
                        ys = ysbp.tile([128, 65], f32, tag="ysb", name="ysb")
                        nc.vector.tensor_copy(ys[:], yh[hf][:])
                        nc.gpsimd.normalize_recip(
                            yq[2 * s + hf][:, 64 * h:64 * h + 64],
                            ys[:, 0:64], ys[:, 64:65])
                    if after_head is not None:
                        after_head(h)

            # ---------------- projections (phase 1+2) ----------------
            with (
                tc.tile_pool(name="wp", bufs=1) as wp,
                tc.tile_pool(name="psA", bufs=1, space="PSUM") as psA,
                tc.tile_pool(name="psB", bufs=1, space="PSUM") as psB,
                tc.tile_pool(name="xk", bufs=2) as xkp,
            ):
                def load_w(param, chunks, cols, tag):
                    tiles = []
                    for i, (k0, kl) in enumerate(chunks):
                        t = wp.tile([128, cols], bf16, tag=f"{tag}{i}",
                                    name=f"{tag}{i}")
                        nc.sync.dma_start(t[:kl, :], param[k0:k0 + kl, :])
                        tiles.append(t)
                    return tiles

                def load_x03(tiles, win, xo):
                    for i, (k0, kl) in enumerate(CCX):
                        nc.sync.dma_start(
                            tiles[i][:kl, xo:xo + 512],
                            xkT[k0:k0 + kl, 512 * win:512 * (win + 1)])

                pjc = [0]

                def pj(w_r, mi, xk_r, xo):
                    # alternate psA/psB per emitted group so bufs=1 reuse is
                    # hidden behind the intervening group's matmuls
                    pool, tag = ((psA, "pja") if pjc[0] % 2 == 0
                                 else (psB, "pjb"))
                    pjc[0] += 1
                    ps = pool.tile([128, 512], f32, tag=tag, name=tag)
                    for ci, (k0, kl) in enumerate(CCQ):
                        nc.tensor.matmul(
                            ps[:, :],
                            w_r[ci][:kl, 128 * mi:128 * (mi + 1)],
                            xk_r[ci][:kl, xo:xo + 512],
                            start=(ci == 0), stop=(ci == 4))
                    return ps

                def kproj_a1(win, xk_r, st, xo=0):
                    # wkks cols: [K g0,g1 | K g2, Ks g0 | Ks g1, Ks g2].
                    # Swapped-side muls are written cross-base so each add's
                    # two inputs share a base partition (verifier rule).
                    c0 = 512 * win
                    ps0 = pj(wk_r, 0, xk_r, xo)
                    t1a = rp.tile([128, 512], bf16, tag="kt1a", name="kt1a")
                    nc.vector.tensor_mul(t1a[:], ps0[:], c2k_t[:, c0:c0 + 512])
                    st.update(t1a=t1a)

                def kproj_a2(win, xk_r, st, xo=0):
                    c0 = 512 * win
                    ps1 = pj(wk_r, 1, xk_r, xo)
                    t1b = rp.tile([64, 512], bf16, tag="kt1b", name="kt1b")
                    t2b = rp.tile([64, 512], bf16, tag="kt2b", name="kt2b")
                    nc.vector.tensor_mul(t1b[:], ps1[0:64, :],
                                         c2k_t[0:64, c0:c0 + 512])
                    nc.vector.tensor_mul(t2b[0:64, :], ps1[64:128, :],
                                         s2k_t[64:128, c0:c0 + 512])
                    st.update(t1b=t1b, t2b=t2b)

                def kproj_a(win, xk_r, st, xo=0):
                    kproj_a1(win, xk_r, st, xo)
                    kproj_a2(win, xk_r, st, xo)

                def kproj_b(win, xk_r, st, xo=0):
                    kdone.add(win)
                    c0 = 512 * win
                    ps2 = pj(wk_r, 2, xk_r, xo)
                    t2a = rp.tile([128, 512], bf16, tag="kt2a", name="kt2a")
                    nc.vector.tensor_mul(t2a[64:128, :], ps2[0:64, :],
                                         s2k_t[0:64, c0:c0 + 512])
                    nc.vector.tensor_mul(t2a[0:64, :], ps2[64:128, :],
                                         s2k_t[64:128, c0:c0 + 512])
                    t1a, t1b, t2b = st["t1a"], st["t1b"], st["t2b"]
                    nc.gpsimd.tensor_add(kt_h[0][0:64, c0:c0 + 512],
                                         t1a[0:64, :], t2b[0:64, :])
                    nc.gpsimd.tensor_add(kt_h[1][0:64, c0:c0 + 512],
                                         t1a[64:128, :], t2a[64:128, :])
                    nc.gpsimd.tensor_add(kt_h[2][0:64, c0:c0 + 512],
                                         t1b[0:64, :], t2a[0:64, :])


                def vproj(win, xk_r, ti, xo=0):
                    t_ = 4 * win + ti
                    vdone.add(t_)
                    pool, tag = ((psA, "pja") if pjc[0] % 2 == 0
                                 else (psB, "pjb"))
                    pjc[0] += 1
                    ps = pool.tile([128, 512], f32, tag=tag, name=tag)
                    for ci, (k0, kl) in enumerate(CCX):
                        nc.tensor.matmul(
                            ps[:, 0:195],
                            xk_r[ci][:kl, xo + 128 * ti:xo + 128 * (ti + 1)],
                            wv_r[ci][:kl, :],
                            start=(ci == 0), stop=(ci == 4))
                    nc.vector.tensor_copy(v_t[t_][:], ps[:, 0:195])

                def qproj_mi(win, xk_r, st, mi, xo=0):
                    # wqqs cols: [Q h0..h8 | Qs h0..h8]. Qs h sits at col
                    # 576+64h (opposite 64-parity to Q h), so swapped muls
                    # write cross-base to align each add's input pair.
                    c0 = 512 * win
                    ps = pj(wq_r, mi, xk_r, xo)
                    if mi <= 3:
                        t = rp.tile([128, 512], bf16, tag=f"qt1_{mi}",
                                    name=f"qt1_{mi}")
                        nc.vector.tensor_mul(t[:], ps[:],
                                             c2k_t[:, c0:c0 + 512])
                        st[("t1", mi)] = t
                    elif mi == 4:
                        ta = rp.tile([64, 512], bf16, tag="qt1_4",
                                     name="qt1_4")
                        tb = rp.tile([64, 512], bf16, tag="qt2_4",
                                     name="qt2_4")
                        nc.vector.tensor_mul(ta[:], ps[0:64, :],
                                             c2k_t[0:64, c0:c0 + 512])
                        # Qs h0 at rows 64:128 -> base 0
                        nc.vector.tensor_mul(tb[0:64, :], ps[64:128, :],
                                             s2k_t[64:128, c0:c0 + 512])
                        st[("t1", 4)], st[("t2", 4)] = ta, tb
                    else:
                        t = rp.tile([128, 512], bf16, tag=f"qt2_{mi}",
                                    name=f"qt2_{mi}")
                        # rows 0:64 hold Qs h(odd-src), cross-based
                        nc.vector.tensor_mul(t[64:128, :], ps[0:64, :],
                                             s2k_t[0:64, c0:c0 + 512])
                        nc.vector.tensor_mul(t[0:64, :], ps[64:128, :],
                                             s2k_t[64:128, c0:c0 + 512])
                        st[("t2", mi)] = t

                def qadd(win, st, h):
                    qdone[win].add(h)
                    c0 = 512 * win
                    bd = 64 * (h % 2)
                    a = st[("t1", h // 2)]
                    b = st[("t2", (576 + 64 * h) // 128)]
                    nc.gpsimd.tensor_add(
                        qth[h][0:64, c0:c0 + 512],
                        a[bd:bd + 64, :], b[bd:bd + 64, :])

                # Q emission order: mi pairs that enable per-head adds ASAP
                Q_SEQ = [("q", 0), ("q", 4), ("a", 0),
                         ("q", 1), ("q", 5), ("a", 1), ("a", 2),
                         ("q", 2), ("q", 6), ("a", 3), ("a", 4),
                         ("q", 3), ("q", 7), ("a", 5), ("a", 6),
                         ("q", 8), ("a", 7), ("a", 8)]
                Q_UNITS = [[("q", 1)], [("q", 5)], [("a", 1), ("a", 2)],
                           [("q", 2)], [("q", 6)], [("a", 3), ("a", 4)],
                           [("q", 3)], [("q", 7)], [("a", 5), ("a", 6)],
                           [("q", 8)], [("a", 7), ("a", 8)]]

                def q_emit(win, xk_r, st, items, xo=0):
                    for kind, v in items:
                        if kind == "q":
                            qproj_mi(win, xk_r, st, v, xo)
                        else:
                            qadd(win, st, v)

                # DMA order: HWDGE generates one DMA per ~625ns, so emit in
                # consumption order: wk+xk1+xk2 interleaved, tables, wv, wq;
                # bulky late-needed constants (masks, iden, wo) afterwards.
                wk_r, xw12 = [], []
                for i, (k0, kl) in enumerate(CCQ):
                    t = wp.tile([128, 2 * HKV * D], bf16, tag=f"wk{i}",
                                name=f"wk{i}")
                    nc.sync.dma_start(t[:kl, :], wkks[k0:k0 + kl, :])
                    wk_r.append(t)
                    k0x, klx = CCX[i]
                    tx = xkp.tile([128, 1024], bf16, tag=f"xw12_{i}",
                                  name=f"xw12_{i}")
                    nc.sync.dma_start(tx[:klx, :], xkT[k0x:k0x + klx, 512:1536])
                    xw12.append(tx)
                nc.sync.dma_start(c2k_t[:], c2k[:])
                nc.sync.dma_start(s2k_t[:], s2k[:])
                wv_r = load_w(wvp, CCX, 195, "wv")
                wq_r = load_w(wqqs, CCQ, 2 * C, "wq")
                xw03 = [xkp.tile([128, 1024], bf16, tag=f"xw03_{i}",
                                 name=f"xw03_{i}") for i in range(5)]
                xk1, xk2 = xw12, xw12

                # Front: K win1, K win2, Q win1 (eager adds; slot-3 scores
                # can start right after h0's add), then the V chunks slot 3
                # needs (6,7,8,9). Scores only gate on kt/qth; PV waits V a
                # little longer without stalling the exp stream.
                stk1, stk2, stq1 = {}, {}, {}
                kproj_a(1, xw12, stk1, 0)
                kproj_b(1, xw12, stk1, 0)
                kproj_a(2, xw12, stk2, 512)
                kproj_b(2, xw12, stk2, 512)
                q_emit(1, xw12, stq1, Q_SEQ[0:3], 0)
                vproj(1, xw12, 2, 0)
                vproj(1, xw12, 3, 0)
                vproj(2, xw12, 0, 512)
                vproj(2, xw12, 1, 512)
                nc.sync.dma_start(m_b[:], masksp[:])
                nc.sync.dma_start(id_t[:], idenp[:])
                for i, (k0, kl) in enumerate(MM):
                    nc.sync.dma_start(wo_r[i][:kl, :], woT[k0:k0 + kl, :])
                for items in Q_UNITS:
                    fillers.append(
                        (lambda it: lambda: q_emit(1, xw12, stq1, it, 0))
                        (items))
                fillers.append(lambda: vproj(2, xw12, 2, 512))
                fillers.append(lambda: vproj(2, xw12, 3, 512))
                fillers.append(lambda: vproj(1, xw12, 0, 0))
                fillers.append(lambda: vproj(1, xw12, 1, 0))

                def win_filler(win, with_q):
                    st = {}
                    xo = 0 if win == 0 else 512

                    def f_load():
                        load_x03(xw03, win, xo)

                    def f_ka1():
                        kproj_a1(win, xw03, st, xo)

                    def f_ka2():
                        kproj_a2(win, xw03, st, xo)

                    def f_kb():
                        kproj_b(win, xw03, st, xo)

                    def f_v(ti):
                        return lambda: vproj(win, xw03, ti, xo)

                    def f_qs(items):
                        return lambda: q_emit(win, xw03, st, items, xo)

                    units = [f_load, f_ka1, f_ka2, f_v(0), f_kb,
                             f_v(1), f_v(2), f_v(3)]
                    if with_q:
                        units += [f_qs(Q_SEQ[0:2]), f_qs(Q_SEQ[2:3])]
                        units += [f_qs(it) for it in Q_UNITS]
                    return units

                fillers.extend(win_filler(0, True))

                attn_slot(3, pump_plan=[2, 2, 2, 1, 1, 1, 1, 1, 1])
                fillers.extend(win_filler(3, False))
                attn_slot(2, pump_plan=[2] * 16 + [1] * 2)
                while fillers:
                    pump()

            # ---------------- transposes, out-proj, last slot ----------------
            with (
                tc.tile_pool(name="psT", bufs=1, space="PSUM") as psT,
                tc.tile_pool(name="psR", bufs=1, space="PSUM") as psR,
            ):
                def transp_block(qc, m):
                    def g():
                        mc0, mrows = MM[m]
                        pt = psT.tile([128, 128], bf16, tag="pt", name="pt",
                                      padded_shape=[128, 1024])
                        nc.tensor.matmul(pt[:mrows, :],
                                         yq[qc][:, mc0:mc0 + mrows],
                                         id_t[:], start=True, stop=True,
                                         is_transpose=True)
                        nc.vector.tensor_copy(
                            ypr[m][0:mrows, 128 * qc:128 * (qc + 1)],
                            pt[:mrows, :])
                    return g

                ost_cur = {}

                def oproj_m(qq, m, pool=None):
                    # qq = 256-col q-quarter; quarter 2s/2s+1 ready as soon
                    # as slot s is transposed, so most of oproj hides inside
                    # the remaining attention stream. The tail quarter
                    # ping-pongs psR with the (idle by then) psS pool. The 5
                    # m-chunk results stage into one [128, 1280] tile and
                    # leave as a single [576, 256] DMA.
                    def g():
                        mc0, mrows = MM[m]
                        po = pool if pool is not None else psR
                        tg = "scores" if po is psS else "pjr"
                        shp = [128, 1024] if po is psS else [128, 256]
                        ps = po.tile(shp, f32, tag=tg, name=tg,
                                     padded_shape=[128, 512] if po is psR
                                     else None)
                        for p, (pc0, pl) in enumerate(MM):
                            nc.tensor.matmul(
                                ps[:mrows, 0:256],
                                wo_r[p][:pl, mc0:mc0 + mrows],
                                ypr[p][:pl, 256 * qq:256 * (qq + 1)],
                                start=(p == 0), stop=(p == 4))
                        if m == 0:
                            ost_cur[qq] = ostp.tile(
                                [128, 1280], f32, tag="ostage", name="ostage")
                        ost = ost_cur[qq]
                        nc.vector.tensor_copy(
                            ost[:mrows, 256 * m:256 * m + 256],
                            ps[:mrows, 0:256])
                        if m == 4:
                            nc.sync.dma_start(
                                yT[:, :, 256 * qq:256 * (qq + 1)],
                                ost[:, :].rearrange("p (m q) -> p m q", m=5))
                    return g

                for m in range(5):
                    fillers.append(transp_block(6, m))
                    fillers.append(transp_block(7, m))
                fillers.extend([oproj_m(3, m) for m in range(5)])
                for m in range(5):
                    fillers.append(transp_block(4, m))
                    fillers.append(transp_block(5, m))
                attn_slot(1)
                fillers.extend([oproj_m(2, m) for m in range(5)])
                for m in range(5):
                    fillers.append(transp_block(2, m))
                    fillers.append(transp_block(3, m))
                fillers.extend([oproj_m(1, m) for m in range(5)])

                def slot0_hook(h):
                    # yq[0]/yq[1] cols for feature-chunk m complete once
                    # heads 2m and 2m+1 have divided; transpose them now so
                    # only oproj qq0 remains after the slot.
                    if h % 2 == 1 and h >= 1:
                        m = (h - 1) // 2
                        fillers.append(transp_block(0, m))
                        fillers.append(transp_block(1, m))
                    elif h == 8:
                        fillers.append(transp_block(0, 4))
                        fillers.append(transp_block(1, 4))

                attn_slot(0, after_head=slot0_hook)
                while fillers:
                    pump()
                for m in range(5):
                    oproj_m(0, m, psS if m % 2 == 1 else psR)()

    nc.compile()
    return nc


def _get_program():
    global _PROG
    if _PROG is None:
        _PROG = _build_program()
    return _PROG


def _neox_perm(nheads, swap=False):
    p = []
    for h in range(nheads):
        ev = [64 * h + 2 * j for j in range(32)]
        od = [64 * h + 2 * j + 1 for j in range(32)]
        p += (od + ev) if swap else (ev + od)
    return np.array(p)


_CONSTS = None


def _static_consts():
    """Input-independent per-core constants (tables, masks, key orders)."""
    global _CONSTS
    if _CONSTS is not None:
        return _CONSTS
    invf = THETA ** (-np.arange(32, dtype=np.float64) / 32)

    def tables(pos):
        ang = pos[None, :] * invf[:, None]
        cos, sin = np.cos(ang), np.sin(ang)
        c2 = np.tile(cos, (4, 1)).astype(np.float32)
        s2 = np.tile(np.vstack([-sin, sin]), (2, 1)).astype(np.float32)
        return c2, s2

    per_j = []
    for j in range(2):
        keypos = np.concatenate(
            [np.arange(QB * q, QB * (q + 1)) for q in KEYORDER[j]])
        qsel = keypos[:TQ]          # queries = first 1024 permuted keys
        c2k, s2k = tables(keypos.astype(np.float64))
        masks = np.zeros((16 * 128, QB), np.float32)
        for s in range(4):
            seq = _slot_seq(s)
            qpos = keypos[QB * s:QB * (s + 1)]
            for k in range(4):
                c = seq[-4 + k]
                kpos = keypos[128 * c:128 * (c + 1)]
                masks[(4 * s + k) * 128:(4 * s + k + 1) * 128] = (
                    kpos[:, None] <= qpos[None, :]).astype(np.float32)
        # device layout: [128, 16*QB] (16 chunk-masks side by side)
        masks2 = masks.reshape(16, 128, QB).transpose(1, 0, 2).reshape(128, 16 * QB)
        per_j.append((keypos, qsel,
                      c2k.astype(ml_dtypes.bfloat16),
                      s2k.astype(ml_dtypes.bfloat16),
                      masks2.astype(ml_dtypes.bfloat16)))
    _CONSTS = per_j
    return _CONSTS


def _host_prep(x, Wq, Wk, Wv, Wo):
    bf = ml_dtypes.bfloat16
    wqqs = np.hstack([Wq[_neox_perm(H)].T,
                      Wq[_neox_perm(H, swap=True)].T]).astype(bf)
    wkks = np.hstack([Wk[_neox_perm(HKV)].T,
                      Wk[_neox_perm(HKV, swap=True)].T]).astype(bf)
    woT = Wo.T.astype(bf)
    wvp = np.zeros((577, 195), np.float32)
    for g in range(HKV):
        wvp[:C, 65 * g:65 * g + 64] = Wv[64 * g:64 * g + 64].T
        wvp[576, 65 * g + 64] = 1.0
    wvp = wvp.astype(bf)
    iden = np.eye(128, dtype=np.float32).astype(bf)

    per_j = _static_consts()
    ones = np.ones((1, T), np.float32)
    in_maps = []
    core_meta = []
    for b in range(B):
        xbT = x[b].T
        for j in range(2):
            keypos, qsel, c2k, s2k, masks = per_j[j]
            xkT = np.vstack([xbT[:, keypos], ones]).astype(bf)
            in_maps.append({
                "xkT": xkT,
                "wqqs": wqqs, "wkks": wkks, "wvp": wvp, "woT": woT,
                "c2k": c2k, "s2k": s2k,
                "masks": masks, "iden": iden,
            })
            core_meta.append((b, qsel))
    return in_maps, core_meta


def kernel(x, Wq, Wk, Wv, Wo):
    x = np.asarray(x, np.float32)
    Wq = np.asarray(Wq, np.float32)
    Wk = np.asarray(Wk, np.float32)
    Wv = np.asarray(Wv, np.float32)
    Wo = np.asarray(Wo, np.float32)

    from concourse.bass_utils import run_bass_kernel_spmd

    nc = _get_program()
    in_maps, core_meta = _host_prep(x, Wq, Wk, Wv, Wo)
    res = run_bass_kernel_spmd(nc, in_maps, list(range(8)))

    out = np.empty((B, T, C), np.float32)
    for core, (b, qsel) in enumerate(core_meta):
        y2 = np.asarray(res.results[core]["yT"])      # [128, 5, TQ]
        yfull = np.moveaxis(y2, 1, 0).reshape(640, TQ)[0:C]
        out[b, qsel, :] = yfull.T
    return out


# revision 21
# speedup vs baseline: 1.0021x; 1.0021x over previous
"""Trainium2 Bass kernel for CausalSelfAttention (RoPE + GQA), 8-core SPMD.

Sharding: 8 cores = 4 batches x 2 query-halves (as v1). Keys PERMUTED per
core so slot s consumes the static key-chunk range [2s, 2s+PAD_s); the first
1024 permuted keys ARE the core's queries, so Q projection reuses the same
x input and RoPE tables.

v2 changes vs v1 (236us):
  - All projections in bf16 (PE cost is free-dim rows only; bf16 allows
    free<256 at full rate and halves DMA). RoPE still via double projection
    (normal + pair-swapped weights), but packed into single weight matrices
    (Q+Qs = 1152 = 9x128 cols, K+Ks = 384 = 3x128) so no 64-row matmuls.
  - PV transposed: out[q,65] = P_chunk^T @ V[keys,65] with bf16 V moving
    (free 65 vs 256 -> PV PE cost halved); ones-column gives the softmax
    denominator per q-PARTITION, so the divide is a per-partition scalar op
    (gpsimd normalize_recip) instead of reciprocal+partition_broadcast+mul.
  - y [q, feat] transposed back for the output projection with PE bf16
    transposes via an identity matrix (cheap: 128 rows each).
  - exp -> bf16 P; mask multiply all-bf16 on DVE (2x mode).
  - Projection/transpose/oproj work is emitted through a filler queue
    interleaved between attention heads so PE fills the gaps of the
    ACT(exp)-paced attention stream.
"""
import sys

sys.path.insert(0, "/opt/trn_rl_repo")

import numpy as np
import ml_dtypes

B, T, C = 4, 2048, 576
H, HKV, D = 9, 3, 64
THETA = 10000.0
QB = 256                      # query block
TQ = 1024                     # queries per core
SLOT_PAD = [16, 12, 8, 4]     # padded key-chunk counts per slot
QBLOCKS = [[7, 5, 2, 0], [6, 4, 3, 1]]   # q-256-block ids per half j
KEYORDER = [[7, 5, 2, 0, 1, 3, 4, 6], [6, 4, 3, 1, 0, 2, 5, 7]]
CCX = [(0, 128), (128, 128), (256, 128), (384, 128), (512, 65)]   # 577 rows incl ones
CCQ = [(0, 128), (128, 128), (256, 128), (384, 128), (512, 64)]   # 576-row chunks
MM = [(0, 128), (128, 128), (256, 128), (384, 128), (512, 64)]    # 576 out chunks


def _slot_seq(s):
    """Key-chunk emission order for slot s: fulls, then the two diag chunks."""
    return list(range(2 * s + 2, 2 * s + SLOT_PAD[s])) + [2 * s, 2 * s + 1]


_PROG = None


def _build_program():
    import concourse.bacc as bacc
    import concourse.mybir as mybir
    import concourse.tile as tile

    dt = mybir.dt
    f32, bf16, fp8 = dt.float32, dt.bfloat16, dt.float8e4
    AF = mybir.ActivationFunctionType

    nc = bacc.Bacc("TRN2", target_bir_lowering=False, debug=False, num_devices=8)

    def inp(name, shape, d):
        return nc.declare_dram_parameter(name, shape, d, isOutput=False)

    xkT = inp("xkT", [577, T], bf16)
    wqk = inp("wqk", [C, 2 * C + 2 * HKV * D], bf16)
    wvp = inp("wvp", [577, 195], bf16)
    woT = inp("woT", [C, C], bf16)
    c2k = inp("c2k", [128, T], bf16)
    s2k = inp("s2k", [128, T], bf16)
    masksp = inp("masks", [128, 16 * QB], bf16)
    idenp = inp("iden", [128, 128], bf16)
    # output in partition-major layout [128, 5, TQ]: row f=128m+p of the
    # logical [576, TQ] lives at [p, m, :]; host reassembles.
    yT = nc.declare_dram_parameter("yT", [128, 5, TQ], f32, isOutput=True)

    with tile.TileContext(nc) as tc:
        with (
            tc.tile_pool(name="const", bufs=1) as cp,
            tc.tile_pool(name="rope", bufs=2) as rp,
            tc.tile_pool(name="pwork", bufs=3) as pw,
            tc.tile_pool(name="ysb", bufs=2) as ysbp,
            tc.tile_pool(name="ost", bufs=2) as ostp,
            tc.tile_pool(name="psS", bufs=2, space="PSUM") as psS,
            tc.tile_pool(name="psY", bufs=2, space="PSUM") as psY,
        ):
            # ---------------- persistent constants ----------------
            # (DMA emission for most constants is deferred into the
            # projection phase so the first window's x/wk loads go first.)
            wo_r = [cp.tile([128, C], bf16, tag=f"wo{i}", name=f"wo{i}")
                    for i in range(5)]
            m_b = cp.tile([128, 16 * QB], bf16, tag="masks", name="masks")
            c2k_t = cp.tile([128, T], bf16, tag="c2k", name="c2k")
            s2k_t = cp.tile([128, T], bf16, tag="s2k", name="s2k")
            id_t = cp.tile([128, 128], bf16, tag="iden", name="iden")

            kt_h = [cp.tile([64, T], bf16, tag=f"kt{g}", name=f"kt{g}")
                    for g in range(HKV)]
            qth = [cp.tile([64, TQ], bf16, tag=f"qth{h}", name=f"qth{h}")
                   for h in range(H)]
            v_t = [cp.tile([128, 195], bf16, tag=f"v{c}", name=f"v{c}")
                   for c in range(16)]
            yq = [cp.tile([128, C], bf16, tag=f"yq{q}", name=f"yq{q}")
                  for q in range(8)]
            ypr = [cp.tile([128, TQ], bf16, tag=f"ypr{p}", name=f"ypr{p}")
                   for p in range(5)]

            # ---------------- attention ----------------
            fillers = []
            qdone = {0: set(), 1: set()}
            kdone = set()
            vdone = set()

            def pump(k=1):
                for _ in range(k):
                    if fillers:
                        fillers.pop(0)()

            def attn_slot(s, after_head=None, pump_k=1, pump_plan=None):
                seq = _slot_seq(s)
                n = len(seq)
                site = [0]
                win = 1 if s >= 2 else 0
                for h in range(H):
                    g = h // 3
                    yh = [psY.tile([128, 65], f32, tag="ypsum", name="ypsum",
                                   padded_shape=[128, 512]) for _ in range(2)]
                    for sc in range(n // 4):
                        grp = seq[4 * sc:4 * sc + 4]
                        while (h not in qdone[win]
                               or any(c not in vdone for c in grp)
                               or any((c // 4) not in kdone for c in grp)):
                            assert fillers, (
                                f"slot {s} head {h} group {sc} deps not in "
                                f"filler queue")
                            fillers.pop(0)()
                        sp = psS.tile([128, 4 * QB], f32, tag="scores",
                                      name="scores")
                        for i in range(4):
                            c = seq[4 * sc + i]
                            nc.tensor.matmul(
                                sp[:, QB * i:QB * (i + 1)],
                                kt_h[g][0:64, 128 * c:128 * (c + 1)],
                                qth[h][0:64, QB * s:QB * (s + 1)],
                                start=True, stop=True)
                        p_b = pw.tile([128, 4 * QB], bf16, tag="p", name="p")
                        nc.scalar.activation(p_b[:], sp[:], AF.Exp, scale=0.125)
                        if sc == n // 4 - 1:
                            nc.vector.tensor_mul(
                                p_b[:], p_b[:],
                                m_b[:, 1024 * s:1024 * (s + 1)])
                        for i in range(4):
                            c = seq[4 * sc + i]
                            ci = 4 * sc + i
                            for hf in range(2):
                                nc.tensor.matmul(
                                    yh[hf][:, 0:65],
                                    p_b[:, QB * i + 128 * hf:
                                        QB * i + 128 * hf + 128],
                                    v_t[c][:, 65 * g:65 * g + 65],
                                    start=(ci == 0), stop=(ci == n - 1))
                        if pump_plan is not None:
                            k = (pump_plan[site[0]]
                                 if site[0] < len(pump_plan) else 1)
                            site[0] += 1
                            pump(k)
                        else:
                            pump(pump_k)
                    for hf in range(2):
                        ys = ysbp.tile([128, 65], f32, tag="ysb", name="ysb")
                        nc.vector.tensor_copy(ys[:], yh[hf][:])
                        nc.gpsimd.normalize_recip(
                            yq[2 * s + hf][:, 64 * h:64 * h + 64],
                            ys[:, 0:64], ys[:, 64:65])
                    if after_head is not None:
                        after_head(h)

            # ---------------- projections (phase 1+2) ----------------
            with (
                tc.tile_pool(name="wp", bufs=1) as wp,
                tc.tile_pool(name="psA", bufs=1, space="PSUM") as psA,
                tc.tile_pool(name="psB", bufs=1, space="PSUM") as psB,
                tc.tile_pool(name="xk", bufs=2) as xkp,
            ):
                def load_w(param, chunks, cols, tag):
                    tiles = []
                    for i, (k0, kl) in enumerate(chunks):
                        t = wp.tile([128, cols], bf16, tag=f"{tag}{i}",
                                    name=f"{tag}{i}")
                        nc.sync.dma_start(t[:kl, :], param[k0:k0 + kl, :])
                        tiles.append(t)
                    return tiles

                def load_x03(tiles, win, xo):
                    for i, (k0, kl) in enumerate(CCX):
                        nc.sync.dma_start(
                            tiles[i][:kl, xo:xo + 512],
                            xkT[k0:k0 + kl, 512 * win:512 * (win + 1)])

                pjc = [0]

                def pj(wbase, mi, xk_r, xo):
                    # alternate psA/psB per emitted group so bufs=1 reuse is
                    # hidden behind the intervening group's matmuls
                    # wbase: (tiles, column offset) into the combined wqk
                    pool, tag = ((psA, "pja") if pjc[0] % 2 == 0
                                 else (psB, "pjb"))
                    pjc[0] += 1
                    wt, wo_ = wbase
                    ps = pool.tile([128, 512], f32, tag=tag, name=tag)
                    for ci, (k0, kl) in enumerate(CCQ):
                        nc.tensor.matmul(
                            ps[:, :],
                            wt[ci][:kl, wo_ + 128 * mi:wo_ + 128 * (mi + 1)],
                            xk_r[ci][:kl, xo:xo + 512],
                            start=(ci == 0), stop=(ci == 4))
                    return ps

                def kproj_a1(win, xk_r, st, xo=0):
                    # wkks cols: [K g0,g1 | K g2, Ks g0 | Ks g1, Ks g2].
                    # Swapped-side muls are written cross-base so each add's
                    # two inputs share a base partition (verifier rule).
                    c0 = 512 * win
                    ps0 = pj(wk_b, 0, xk_r, xo)
                    t1a = rp.tile([128, 512], bf16, tag="kt1a", name="kt1a")
                    nc.vector.tensor_mul(t1a[:], ps0[:], c2k_t[:, c0:c0 + 512])
                    st.update(t1a=t1a)

                def kproj_a2(win, xk_r, st, xo=0):
                    c0 = 512 * win
                    ps1 = pj(wk_b, 1, xk_r, xo)
                    t1b = rp.tile([64, 512], bf16, tag="kt1b", name="kt1b")
                    t2b = rp.tile([64, 512], bf16, tag="kt2b", name="kt2b")
                    nc.vector.tensor_mul(t1b[:], ps1[0:64, :],
                                         c2k_t[0:64, c0:c0 + 512])
                    nc.vector.tensor_mul(t2b[0:64, :], ps1[64:128, :],
                                         s2k_t[64:128, c0:c0 + 512])
                    st.update(t1b=t1b, t2b=t2b)

                def kproj_a(win, xk_r, st, xo=0):
                    kproj_a1(win, xk_r, st, xo)
                    kproj_a2(win, xk_r, st, xo)

                def kproj_b(win, xk_r, st, xo=0):
                    kdone.add(win)
                    c0 = 512 * win
                    ps2 = pj(wk_b, 2, xk_r, xo)
                    t2a = rp.tile([128, 512], bf16, tag="kt2a", name="kt2a")
                    nc.vector.tensor_mul(t2a[64:128, :], ps2[0:64, :],
                                         s2k_t[0:64, c0:c0 + 512])
                    nc.vector.tensor_mul(t2a[0:64, :], ps2[64:128, :],
                                         s2k_t[64:128, c0:c0 + 512])
                    t1a, t1b, t2b = st["t1a"], st["t1b"], st["t2b"]
                    nc.gpsimd.tensor_add(kt_h[0][0:64, c0:c0 + 512],
                                         t1a[0:64, :], t2b[0:64, :])
                    nc.gpsimd.tensor_add(kt_h[1][0:64, c0:c0 + 512],
                                         t1a[64:128, :], t2a[64:128, :])
                    nc.gpsimd.tensor_add(kt_h[2][0:64, c0:c0 + 512],
                                         t1b[0:64, :], t2a[0:64, :])


                def vproj(win, xk_r, ti, xo=0):
                    t_ = 4 * win + ti
                    vdone.add(t_)
                    pool, tag = ((psA, "pja") if pjc[0] % 2 == 0
                                 else (psB, "pjb"))
                    pjc[0] += 1
                    ps = pool.tile([128, 512], f32, tag=tag, name=tag)
                    for ci, (k0, kl) in enumerate(CCX):
                        nc.tensor.matmul(
                            ps[:, 0:195],
                            xk_r[ci][:kl, xo + 128 * ti:xo + 128 * (ti + 1)],
                            wv_r[ci][:kl, :],
                            start=(ci == 0), stop=(ci == 4))
                    nc.vector.tensor_copy(v_t[t_][:], ps[:, 0:195])

                def qproj_mi(win, xk_r, st, mi, xo=0):
                    # wqqs cols: [Q h0..h8 | Qs h0..h8]. Qs h sits at col
                    # 576+64h (opposite 64-parity to Q h), so swapped muls
                    # write cross-base to align each add's input pair.
                    c0 = 512 * win
                    ps = pj(wq_b, mi, xk_r, xo)
                    if mi <= 3:
                        t = rp.tile([128, 512], bf16, tag=f"qt1_{mi}",
                                    name=f"qt1_{mi}")
                        nc.vector.tensor_mul(t[:], ps[:],
                                             c2k_t[:, c0:c0 + 512])
                        st[("t1", mi)] = t
                    elif mi == 4:
                        ta = rp.tile([64, 512], bf16, tag="qt1_4",
                                     name="qt1_4")
                        tb = rp.tile([64, 512], bf16, tag="qt2_4",
                                     name="qt2_4")
                        nc.vector.tensor_mul(ta[:], ps[0:64, :],
                                             c2k_t[0:64, c0:c0 + 512])
                        # Qs h0 at rows 64:128 -> base 0
                        nc.vector.tensor_mul(tb[0:64, :], ps[64:128, :],
                                             s2k_t[64:128, c0:c0 + 512])
                        st[("t1", 4)], st[("t2", 4)] = ta, tb
                    else:
                        t = rp.tile([128, 512], bf16, tag=f"qt2_{mi}",
                                    name=f"qt2_{mi}")
                        # rows 0:64 hold Qs h(odd-src), cross-based
                        nc.vector.tensor_mul(t[64:128, :], ps[0:64, :],
                                             s2k_t[0:64, c0:c0 + 512])
                        nc.vector.tensor_mul(t[0:64, :], ps[64:128, :],
                                             s2k_t[64:128, c0:c0 + 512])
                        st[("t2", mi)] = t

                def qadd(win, st, h):
                    qdone[win].add(h)
                    c0 = 512 * win
                    bd = 64 * (h % 2)
                    a = st[("t1", h // 2)]
                    b = st[("t2", (576 + 64 * h) // 128)]
                    nc.gpsimd.tensor_add(
                        qth[h][0:64, c0:c0 + 512],
                        a[bd:bd + 64, :], b[bd:bd + 64, :])

                # Q emission order: mi pairs that enable per-head adds ASAP
                Q_SEQ = [("q", 0), ("q", 4), ("a", 0),
                         ("q", 1), ("q", 5), ("a", 1), ("a", 2),
                         ("q", 2), ("q", 6), ("a", 3), ("a", 4),
                         ("q", 3), ("q", 7), ("a", 5), ("a", 6),
                         ("q", 8), ("a", 7), ("a", 8)]
                Q_UNITS = [[("q", 1)], [("q", 5)], [("a", 1), ("a", 2)],
                           [("q", 2)], [("q", 6)], [("a", 3), ("a", 4)],
                           [("q", 3)], [("q", 7)], [("a", 5), ("a", 6)],
                           [("q", 8)], [("a", 7), ("a", 8)]]

                def q_emit(win, xk_r, st, items, xo=0):
                    for kind, v in items:
                        if kind == "q":
                            qproj_mi(win, xk_r, st, v, xo)
                        else:
                            qadd(win, st, v)

                # DMA order: HWDGE generates one DMA per ~625ns, so emit in
                # consumption order: wk+xk1+xk2 interleaved, tables, wv, wq;
                # bulky late-needed constants (masks, iden, wo) afterwards.
                wqk_r, xw12 = [], []
                for i, (k0, kl) in enumerate(CCQ):
                    t = wp.tile([128, 2 * C + 2 * HKV * D], bf16,
                                tag=f"wqk{i}", name=f"wqk{i}")
                    nc.sync.dma_start(t[:kl, :], wqk[k0:k0 + kl, :])
                    wqk_r.append(t)
                    k0x, klx = CCX[i]
                    tx = xkp.tile([128, 1024], bf16, tag=f"xw12_{i}",
                                  name=f"xw12_{i}")
                    nc.sync.dma_start(tx[:klx, :], xkT[k0x:k0x + klx, 512:1536])
                    xw12.append(tx)
                nc.sync.dma_start(c2k_t[:], c2k[:])
                nc.sync.dma_start(s2k_t[:], s2k[:])
                wv_r = load_w(wvp, CCX, 195, "wv")
                wq_b = (wqk_r, 0)
                wk_b = (wqk_r, 2 * C)
                xw03 = [xkp.tile([128, 1024], bf16, tag=f"xw03_{i}",
                                 name=f"xw03_{i}") for i in range(5)]
                xk1, xk2 = xw12, xw12

                # Front: K win1, K win2, Q win1 (eager adds; slot-3 scores
                # can start right after h0's add), then the V chunks slot 3
                # needs (6,7,8,9). Scores only gate on kt/qth; PV waits V a
                # little longer without stalling the exp stream.
                stk1, stk2, stq1 = {}, {}, {}
                kproj_a(1, xw12, stk1, 0)
                kproj_b(1, xw12, stk1, 0)
                kproj_a(2, xw12, stk2, 512)
                kproj_b(2, xw12, stk2, 512)
                q_emit(1, xw12, stq1, Q_SEQ[0:3], 0)
                vproj(1, xw12, 2, 0)
                vproj(1, xw12, 3, 0)
                vproj(2, xw12, 0, 512)
                vproj(2, xw12, 1, 512)
                nc.sync.dma_start(m_b[:], masksp[:])
                nc.sync.dma_start(id_t[:], idenp[:])
                for i, (k0, kl) in enumerate(MM):
                    nc.sync.dma_start(wo_r[i][:kl, :], woT[k0:k0 + kl, :])
                for items in Q_UNITS:
                    fillers.append(
                        (lambda it: lambda: q_emit(1, xw12, stq1, it, 0))
                        (items))
                fillers.append(lambda: vproj(2, xw12, 2, 512))
                fillers.append(lambda: vproj(2, xw12, 3, 512))
                fillers.append(lambda: vproj(1, xw12, 0, 0))
                fillers.append(lambda: vproj(1, xw12, 1, 0))

                def win_filler(win, with_q):
                    st = {}
                    xo = 0 if win == 0 else 512

                    def f_load():
                        load_x03(xw03, win, xo)

                    def f_ka1():
                        kproj_a1(win, xw03, st, xo)

                    def f_ka2():
                        kproj_a2(win, xw03, st, xo)

                    def f_kb():
                        kproj_b(win, xw03, st, xo)

                    def f_v(ti):
                        return lambda: vproj(win, xw03, ti, xo)

                    def f_qs(items):
                        return lambda: q_emit(win, xw03, st, items, xo)

                    units = [f_load, f_ka1, f_ka2, f_v(0), f_kb,
                             f_v(1), f_v(2), f_v(3)]
                    if with_q:
                        units += [f_qs(Q_SEQ[0:2]), f_qs(Q_SEQ[2:3])]
                        units += [f_qs(it) for it in Q_UNITS]
                    return units

                fillers.extend(win_filler(0, True))

                attn_slot(3)
                fillers.extend(win_filler(3, False))
                attn_slot(2, pump_plan=[2] * 18)
                while fillers:
                    pump()

            # ---------------- transposes, out-proj, last slot ----------------
            with (
                tc.tile_pool(name="psT", bufs=1, space="PSUM") as psT,
                tc.tile_pool(name="psR", bufs=1, space="PSUM") as psR,
            ):
                def transp_block(qc, m):
                    def g():
                        mc0, mrows = MM[m]
                        pt = psT.tile([128, 128], bf16, tag="pt", name="pt",
                                      padded_shape=[128, 1024])
                        nc.tensor.matmul(pt[:mrows, :],
                                         yq[qc][:, mc0:mc0 + mrows],
                                         id_t[:], start=True, stop=True,
                                         is_transpose=True)
                        nc.vector.tensor_copy(
                            ypr[m][0:mrows, 128 * qc:128 * (qc + 1)],
                            pt[:mrows, :])
                    return g

                ost_cur = {}

                def oproj_m(qq, m, pool=None):
                    # qq = 256-col q-quarter; quarter 2s/2s+1 ready as soon
                    # as slot s is transposed, so most of oproj hides inside
                    # the remaining attention stream. The tail quarter
                    # ping-pongs psR with the (idle by then) psS pool. The 5
                    # m-chunk results stage into one [128, 1280] tile and
                    # leave as a single [576, 256] DMA.
                    def g():
                        mc0, mrows = MM[m]
                        po = pool if pool is not None else psR
                        tg = "scores" if po is psS else "pjr"
                        shp = [128, 1024] if po is psS else [128, 256]
                        ps = po.tile(shp, f32, tag=tg, name=tg,
                                     padded_shape=[128, 512] if po is psR
                                     else None)
                        for p, (pc0, pl) in enumerate(MM):
                            nc.tensor.matmul(
                                ps[:mrows, 0:256],
                                wo_r[p][:pl, mc0:mc0 + mrows],
                                ypr[p][:pl, 256 * qq:256 * (qq + 1)],
                                start=(p == 0), stop=(p == 4))
                        if m == 0:
                            ost_cur[qq] = ostp.tile(
                                [128, 1280], f32, tag="ostage", name="ostage")
                        ost = ost_cur[qq]
                        nc.vector.tensor_copy(
                            ost[:mrows, 256 * m:256 * m + 256],
                            ps[:mrows, 0:256])
                        if m == 4:
                            nc.sync.dma_start(
                                yT[:, :, 256 * qq:256 * (qq + 1)],
                                ost[:, :].rearrange("p (m q) -> p m q", m=5))
                    return g

                for m in range(5):
                    fillers.append(transp_block(6, m))
                    fillers.append(transp_block(7, m))
                fillers.extend([oproj_m(3, m) for m in range(5)])
                for m in range(5):
                    fillers.append(transp_block(4, m))
                    fillers.append(transp_block(5, m))
                attn_slot(1)
                fillers.extend([oproj_m(2, m) for m in range(5)])
                for m in range(5):
                    fillers.append(transp_block(2, m))
                    fillers.append(transp_block(3, m))
                fillers.extend([oproj_m(1, m) for m in range(5)])

                def slot0_hook(h):
                    # yq[0]/yq[1] cols for feature-chunk m complete once
                    # heads 2m and 2m+1 have divided; transpose them now so
                    # only oproj qq0 remains after the slot.
                    if h % 2 == 1 and h >= 1:
                        m = (h - 1) // 2
                        fillers.append(transp_block(0, m))
                        fillers.append(transp_block(1, m))
                    elif h == 8:
                        fillers.append(transp_block(0, 4))
                        fillers.append(transp_block(1, 4))

                attn_slot(0, after_head=slot0_hook)
                while fillers:
                    pump()
                for m in range(5):
                    oproj_m(0, m, psS if m % 2 == 1 else psR)()

    nc.compile()
    return nc


def _get_program():
    global _PROG
    if _PROG is None:
        _PROG = _build_program()
    return _PROG


def _neox_perm(nheads, swap=False):
    p = []
    for h in range(nheads):
        ev = [64 * h + 2 * j for j in range(32)]
        od = [64 * h + 2 * j + 1 for j in range(32)]
        p += (od + ev) if swap else (ev + od)
    return np.array(p)


_CONSTS = None


def _static_consts():
    """Input-independent per-core constants (tables, masks, key orders)."""
    global _CONSTS
    if _CONSTS is not None:
        return _CONSTS
    invf = THETA ** (-np.arange(32, dtype=np.float64) / 32)

    def tables(pos):
        ang = pos[None, :] * invf[:, None]
        cos, sin = np.cos(ang), np.sin(ang)
        c2 = np.tile(cos, (4, 1)).astype(np.float32)
        s2 = np.tile(np.vstack([-sin, sin]), (2, 1)).astype(np.float32)
        return c2, s2

    per_j = []
    for j in range(2):
        keypos = np.concatenate(
            [np.arange(QB * q, QB * (q + 1)) for q in KEYORDER[j]])
        qsel = keypos[:TQ]          # queries = first 1024 permuted keys
        c2k, s2k = tables(keypos.astype(np.float64))
        masks = np.zeros((16 * 128, QB), np.float32)
        for s in range(4):
            seq = _slot_seq(s)
            qpos = keypos[QB * s:QB * (s + 1)]
            for k in range(4):
                c = seq[-4 + k]
                kpos = keypos[128 * c:128 * (c + 1)]
                masks[(4 * s + k) * 128:(4 * s + k + 1) * 128] = (
                    kpos[:, None] <= qpos[None, :]).astype(np.float32)
        # device layout: [128, 16*QB] (16 chunk-masks side by side)
        masks2 = masks.reshape(16, 128, QB).transpose(1, 0, 2).reshape(128, 16 * QB)
        per_j.append((keypos, qsel,
                      c2k.astype(ml_dtypes.bfloat16),
                      s2k.astype(ml_dtypes.bfloat16),
                      masks2.astype(ml_dtypes.bfloat16)))
    _CONSTS = per_j
    return _CONSTS


def _host_prep(x, Wq, Wk, Wv, Wo):
    bf = ml_dtypes.bfloat16
    wqk = np.hstack([Wq[_neox_perm(H)].T,
                     Wq[_neox_perm(H, swap=True)].T,
                     Wk[_neox_perm(HKV)].T,
                     Wk[_neox_perm(HKV, swap=True)].T]).astype(bf)
    woT = Wo.T.astype(bf)
    wvp = np.zeros((577, 195), np.float32)
    for g in range(HKV):
        wvp[:C, 65 * g:65 * g + 64] = Wv[64 * g:64 * g + 64].T
        wvp[576, 65 * g + 64] = 1.0
    wvp = wvp.astype(bf)
    iden = np.eye(128, dtype=np.float32).astype(bf)

    per_j = _static_consts()
    ones = np.ones((1, T), np.float32)
    in_maps = []
    core_meta = []
    for b in range(B):
        xbT = x[b].T
        for j in range(2):
            keypos, qsel, c2k, s2k, masks = per_j[j]
            xkT = np.vstack([xbT[:, keypos], ones]).astype(bf)
            in_maps.append({
                "xkT": xkT,
                "wqk": wqk, "wvp": wvp, "woT": woT,
                "c2k": c2k, "s2k": s2k,
                "masks": masks, "iden": iden,
            })
            core_meta.append((b, qsel))
    return in_maps, core_meta


def kernel(x, Wq, Wk, Wv, Wo):
    x = np.asarray(x, np.float32)
    Wq = np.asarray(Wq, np.float32)
    Wk = np.asarray(Wk, np.float32)
    Wv = np.asarray(Wv, np.float32)
    Wo = np.asarray(Wo, np.float32)

    from concourse.bass_utils import run_bass_kernel_spmd

    nc = _get_program()
    in_maps, core_meta = _host_prep(x, Wq, Wk, Wv, Wo)
    res = run_bass_kernel_spmd(nc, in_maps, list(range(8)))

    out = np.empty((B, T, C), np.float32)
    for core, (b, qsel) in enumerate(core_meta):
        y2 = np.asarray(res.results[core]["yT"])      # [128, 5, TQ]
        yfull = np.moveaxis(y2, 1, 0).reshape(640, TQ)[0:C]
        out[b, qsel, :] = yfull.T
    return out


# revision 24
# speedup vs baseline: 1.0174x; 1.0152x over previous
"""Trainium2 Bass kernel for CausalSelfAttention (RoPE + GQA), 8-core SPMD.

Sharding: 8 cores = 4 batches x 2 query-halves (as v1). Keys PERMUTED per
core so slot s consumes the static key-chunk range [2s, 2s+PAD_s); the first
1024 permuted keys ARE the core's queries, so Q projection reuses the same
x input and RoPE tables.

v2 changes vs v1 (236us):
  - All projections in bf16 (PE cost is free-dim rows only; bf16 allows
    free<256 at full rate and halves DMA). RoPE still via double projection
    (normal + pair-swapped weights), but packed into single weight matrices
    (Q+Qs = 1152 = 9x128 cols, K+Ks = 384 = 3x128) so no 64-row matmuls.
  - PV transposed: out[q,65] = P_chunk^T @ V[keys,65] with bf16 V moving
    (free 65 vs 256 -> PV PE cost halved); ones-column gives the softmax
    denominator per q-PARTITION, so the divide is a per-partition scalar op
    (gpsimd normalize_recip) instead of reciprocal+partition_broadcast+mul.
  - y [q, feat] transposed back for the output projection with PE bf16
    transposes via an identity matrix (cheap: 128 rows each).
  - exp -> bf16 P; mask multiply all-bf16 on DVE (2x mode).
  - Projection/transpose/oproj work is emitted through a filler queue
    interleaved between attention heads so PE fills the gaps of the
    ACT(exp)-paced attention stream.
"""
import sys

sys.path.insert(0, "/opt/trn_rl_repo")

import numpy as np
import ml_dtypes

B, T, C = 4, 2048, 576
H, HKV, D = 9, 3, 64
THETA = 10000.0
QB = 256                      # query block
TQ = 1024                     # queries per core
SLOT_PAD = [16, 12, 8, 4]     # padded key-chunk counts per slot
QBLOCKS = [[7, 5, 2, 0], [6, 4, 3, 1]]   # q-256-block ids per half j
KEYORDER = [[7, 5, 2, 0, 1, 3, 4, 6], [6, 4, 3, 1, 0, 2, 5, 7]]
CCX = [(0, 128), (128, 128), (256, 128), (384, 128), (512, 65)]   # 577 rows incl ones
CCQ = [(0, 128), (128, 128), (256, 128), (384, 128), (512, 64)]   # 576-row chunks
MM = [(0, 128), (128, 128), (256, 128), (384, 128), (512, 64)]    # 576 out chunks


def _slot_seq(s):
    """Key-chunk emission order for slot s: fulls, then the two diag chunks."""
    return list(range(2 * s + 2, 2 * s + SLOT_PAD[s])) + [2 * s, 2 * s + 1]


_PROG = None


def _build_program():
    import concourse.bacc as bacc
    import concourse.mybir as mybir
    import concourse.tile as tile

    dt = mybir.dt
    f32, bf16, fp8 = dt.float32, dt.bfloat16, dt.float8e4
    AF = mybir.ActivationFunctionType

    nc = bacc.Bacc("TRN2", target_bir_lowering=False, debug=False, num_devices=8)

    def inp(name, shape, d):
        return nc.declare_dram_parameter(name, shape, d, isOutput=False)

    xkT = inp("xkT", [577, T], bf16)
    wqk = inp("wqk", [C, 2 * C + 2 * HKV * D], bf16)
    wvp = inp("wvp", [577, 195], bf16)
    woT = inp("woT", [C, C], bf16)
    c2k = inp("c2k", [128, T], bf16)
    s2k = inp("s2k", [128, T], bf16)
    masksp = inp("masks", [128, 16 * QB], bf16)
    idenp = inp("iden", [128, 128], bf16)
    # output in partition-major layout [128, 5, TQ]: row f=128m+p of the
    # logical [576, TQ] lives at [p, m, :]; host reassembles.
    yT = nc.declare_dram_parameter("yT", [128, 5, TQ], f32, isOutput=True)

    with tile.TileContext(nc) as tc:
        with (
            tc.tile_pool(name="const", bufs=1) as cp,
            tc.tile_pool(name="rope", bufs=2) as rp,
            tc.tile_pool(name="pwork", bufs=3) as pw,
            tc.tile_pool(name="ysb", bufs=2) as ysbp,
            tc.tile_pool(name="ost", bufs=2) as ostp,
            tc.tile_pool(name="psS", bufs=2, space="PSUM") as psS,
            tc.tile_pool(name="psY", bufs=2, space="PSUM") as psY,
        ):
            # ---------------- persistent constants ----------------
            # (DMA emission for most constants is deferred into the
            # projection phase so the first window's x/wk loads go first.)
            wo_r = [cp.tile([128, C], bf16, tag=f"wo{i}", name=f"wo{i}")
                    for i in range(5)]
            m_b = cp.tile([128, 16 * QB], bf16, tag="masks", name="masks")
            c2k_t = cp.tile([128, T], bf16, tag="c2k", name="c2k")
            s2k_t = cp.tile([128, T], bf16, tag="s2k", name="s2k")
            id_t = cp.tile([128, 128], bf16, tag="iden", name="iden")

            kt_h = [cp.tile([64, T], bf16, tag=f"kt{g}", name=f"kt{g}")
                    for g in range(HKV)]
            qth = [cp.tile([64, TQ], bf16, tag=f"qth{h}", name=f"qth{h}")
                   for h in range(H)]
            v_t = [cp.tile([128, 195], bf16, tag=f"v{c}", name=f"v{c}")
                   for c in range(16)]
            yq = [cp.tile([128, C], bf16, tag=f"yq{q}", name=f"yq{q}")
                  for q in range(8)]
            ypr = [cp.tile([128, TQ], bf16, tag=f"ypr{p}", name=f"ypr{p}")
                   for p in range(5)]

            # ---------------- attention ----------------
            fillers = []
            qdone = {0: set(), 1: set()}
            kdone = set()
            vdone = set()

            def pump(k=1):
                for _ in range(k):
                    if fillers:
                        fillers.pop(0)()

            def attn_slot(s, after_head=None, pump_k=1, pump_plan=None):
                seq = _slot_seq(s)
                n = len(seq)
                site = [0]
                win = 1 if s >= 2 else 0
                for h in range(H):
                    g = h // 3
                    yh = [psY.tile([128, 65], f32, tag="ypsum", name="ypsum",
                                   padded_shape=[128, 512]) for _ in range(2)]
                    for sc in range(n // 4):
                        grp = seq[4 * sc:4 * sc + 4]
                        while (h not in qdone[win]
                               or any(c not in vdone for c in grp)
                               or any((c // 4) not in kdone for c in grp)):
                            assert fillers, (
                                f"slot {s} head {h} group {sc} deps not in "
                                f"filler queue")
                            fillers.pop(0)()
                        sp = psS.tile([128, 4 * QB], f32, tag="scores",
                                      name="scores")
                        for i in range(4):
                            c = seq[4 * sc + i]
                            nc.tensor.matmul(
                                sp[:, QB * i:QB * (i + 1)],
                                kt_h[g][0:64, 128 * c:128 * (c + 1)],
                                qth[h][0:64, QB * s:QB * (s + 1)],
                                start=True, stop=True)
                        p_b = pw.tile([128, 4 * QB], bf16, tag="p", name="p")
                        nc.scalar.activation(p_b[:], sp[:], AF.Exp, scale=0.125)
                        if sc == n // 4 - 1:
                            nc.vector.tensor_mul(
                                p_b[:], p_b[:],
                                m_b[:, 1024 * s:1024 * (s + 1)])
                        for i in range(4):
                            c = seq[4 * sc + i]
                            ci = 4 * sc + i
                            for hf in range(2):
                                nc.tensor.matmul(
                                    yh[hf][:, 0:65],
                                    p_b[:, QB * i + 128 * hf:
                                        QB * i + 128 * hf + 128],
                                    v_t[c][:, 65 * g:65 * g + 65],
                                    start=(ci == 0), stop=(ci == n - 1))
                        if pump_plan is not None:
                            k = (pump_plan[site[0]]
                                 if site[0] < len(pump_plan) else 1)
                            site[0] += 1
                            pump(k)
                        else:
                            pump(pump_k)
                    for hf in range(2):
                        ys = ysbp.tile([128, 65], f32, tag="ysb", name="ysb")
                        nc.vector.tensor_copy(ys[:], yh[hf][:])
                        nc.gpsimd.normalize_recip(
                            yq[2 * s + hf][:, 64 * h:64 * h + 64],
                            ys[:, 0:64], ys[:, 64:65])
                    if after_head is not None:
                        after_head(h)

            # ---------------- projections (phase 1+2) ----------------
            with (
                tc.tile_pool(name="wp", bufs=1) as wp,
                tc.tile_pool(name="psA", bufs=1, space="PSUM") as psA,
                tc.tile_pool(name="psB", bufs=1, space="PSUM") as psB,
                tc.tile_pool(name="xk", bufs=2) as xkp,
            ):
                def load_w(param, chunks, cols, tag):
                    tiles = []
                    for i, (k0, kl) in enumerate(chunks):
                        t = wp.tile([128, cols], bf16, tag=f"{tag}{i}",
                                    name=f"{tag}{i}")
                        nc.sync.dma_start(t[:kl, :], param[k0:k0 + kl, :])
                        tiles.append(t)
                    return tiles

                def load_x03(tiles, win, xo):
                    for i, (k0, kl) in enumerate(CCX):
                        nc.sync.dma_start(
                            tiles[i][:kl, xo:xo + 512],
                            xkT[k0:k0 + kl, 512 * win:512 * (win + 1)])

                pjc = [0]

                def pj(wbase, mi, xk_r, xo):
                    # alternate psA/psB per emitted group so bufs=1 reuse is
                    # hidden behind the intervening group's matmuls
                    # wbase: (tiles, column offset) into the combined wqk
                    pool, tag = ((psA, "pja") if pjc[0] % 2 == 0
                                 else (psB, "pjb"))
                    pjc[0] += 1
                    wt, wo_ = wbase
                    ps = pool.tile([128, 512], f32, tag=tag, name=tag)
                    for ci, (k0, kl) in enumerate(CCQ):
                        nc.tensor.matmul(
                            ps[:, :],
                            wt[ci][:kl, wo_ + 128 * mi:wo_ + 128 * (mi + 1)],
                            xk_r[ci][:kl, xo:xo + 512],
                            start=(ci == 0), stop=(ci == 4))
                    return ps

                def kproj_a1(win, xk_r, st, xo=0):
                    # wkks cols: [K g0,g1 | K g2, Ks g0 | Ks g1, Ks g2].
                    # Swapped-side muls are written cross-base so each add's
                    # two inputs share a base partition (verifier rule).
                    c0 = 512 * win
                    ps0 = pj(wk_b, 0, xk_r, xo)
                    t1a = rp.tile([128, 512], bf16, tag="kt1a", name="kt1a")
                    nc.vector.tensor_mul(t1a[:], ps0[:], c2k_t[:, c0:c0 + 512])
                    st.update(t1a=t1a)

                def kproj_a2(win, xk_r, st, xo=0):
                    c0 = 512 * win
                    ps1 = pj(wk_b, 1, xk_r, xo)
                    t1b = rp.tile([64, 512], bf16, tag="kt1b", name="kt1b")
                    t2b = rp.tile([64, 512], bf16, tag="kt2b", name="kt2b")
                    nc.vector.tensor_mul(t1b[:], ps1[0:64, :],
                                         c2k_t[0:64, c0:c0 + 512])
                    nc.vector.tensor_mul(t2b[0:64, :], ps1[64:128, :],
                                         s2k_t[64:128, c0:c0 + 512])
                    st.update(t1b=t1b, t2b=t2b)

                def kproj_a(win, xk_r, st, xo=0):
                    kproj_a1(win, xk_r, st, xo)
                    kproj_a2(win, xk_r, st, xo)

                def kproj_b(win, xk_r, st, xo=0):
                    kdone.add(win)
                    c0 = 512 * win
                    ps2 = pj(wk_b, 2, xk_r, xo)
                    t2a = rp.tile([128, 512], bf16, tag="kt2a", name="kt2a")
                    nc.vector.tensor_mul(t2a[64:128, :], ps2[0:64, :],
                                         s2k_t[0:64, c0:c0 + 512])
                    nc.vector.tensor_mul(t2a[0:64, :], ps2[64:128, :],
                                         s2k_t[64:128, c0:c0 + 512])
                    t1a, t1b, t2b = st["t1a"], st["t1b"], st["t2b"]
                    nc.gpsimd.tensor_add(kt_h[0][0:64, c0:c0 + 512],
                                         t1a[0:64, :], t2b[0:64, :])
                    nc.gpsimd.tensor_add(kt_h[1][0:64, c0:c0 + 512],
                                         t1a[64:128, :], t2a[64:128, :])
                    nc.gpsimd.tensor_add(kt_h[2][0:64, c0:c0 + 512],
                                         t1b[0:64, :], t2a[0:64, :])


                def vproj(win, xk_r, ti, xo=0):
                    t_ = 4 * win + ti
                    vdone.add(t_)
                    pool, tag = ((psA, "pja") if pjc[0] % 2 == 0
                                 else (psB, "pjb"))
                    pjc[0] += 1
                    ps = pool.tile([128, 512], f32, tag=tag, name=tag)
                    for ci, (k0, kl) in enumerate(CCX):
                        nc.tensor.matmul(
                            ps[:, 0:195],
                            xk_r[ci][:kl, xo + 128 * ti:xo + 128 * (ti + 1)],
                            wv_r[ci][:kl, :],
                            start=(ci == 0), stop=(ci == 4))
                    nc.vector.tensor_copy(v_t[t_][:], ps[:, 0:195])

                def qproj_mi(win, xk_r, st, mi, xo=0):
                    # wqqs cols: [Q h0..h8 | Qs h0..h8]. Qs h sits at col
                    # 576+64h (opposite 64-parity to Q h), so swapped muls
                    # write cross-base to align each add's input pair.
                    c0 = 512 * win
                    ps = pj(wq_b, mi, xk_r, xo)
                    if mi <= 3:
                        t = rp.tile([128, 512], bf16, tag=f"qt1_{mi}",
                                    name=f"qt1_{mi}")
                        nc.vector.tensor_mul(t[:], ps[:],
                                             c2k_t[:, c0:c0 + 512])
                        st[("t1", mi)] = t
                    elif mi == 4:
                        ta = rp.tile([64, 512], bf16, tag="qt1_4",
                                     name="qt1_4")
                        tb = rp.tile([64, 512], bf16, tag="qt2_4",
                                     name="qt2_4")
                        nc.vector.tensor_mul(ta[:], ps[0:64, :],
                                             c2k_t[0:64, c0:c0 + 512])
                        # Qs h0 at rows 64:128 -> base 0
                        nc.vector.tensor_mul(tb[0:64, :], ps[64:128, :],
                                             s2k_t[64:128, c0:c0 + 512])
                        st[("t1", 4)], st[("t2", 4)] = ta, tb
                    else:
                        t = rp.tile([128, 512], bf16, tag=f"qt2_{mi}",
                                    name=f"qt2_{mi}")
                        # rows 0:64 hold Qs h(odd-src), cross-based
                        nc.vector.tensor_mul(t[64:128, :], ps[0:64, :],
                                             s2k_t[0:64, c0:c0 + 512])
                        nc.vector.tensor_mul(t[0:64, :], ps[64:128, :],
                                             s2k_t[64:128, c0:c0 + 512])
                        st[("t2", mi)] = t

                def qadd(win, st, h):
                    qdone[win].add(h)
                    c0 = 512 * win
                    bd = 64 * (h % 2)
                    a = st[("t1", h // 2)]
                    b = st[("t2", (576 + 64 * h) // 128)]
                    nc.gpsimd.tensor_add(
                        qth[h][0:64, c0:c0 + 512],
                        a[bd:bd + 64, :], b[bd:bd + 64, :])

                # Q emission order: mi pairs that enable per-head adds ASAP
                Q_SEQ = [("q", 0), ("q", 4), ("a", 0),
                         ("q", 1), ("q", 5), ("a", 1), ("a", 2),
                         ("q", 2), ("q", 6), ("a", 3), ("a", 4),
                         ("q", 3), ("q", 7), ("a", 5), ("a", 6),
                         ("q", 8), ("a", 7), ("a", 8)]
                Q_UNITS = [[("q", 1)], [("q", 5)], [("a", 1), ("a", 2)],
                           [("q", 2)], [("q", 6)], [("a", 3), ("a", 4)],
                           [("q", 3)], [("q", 7)], [("a", 5), ("a", 6)],
                           [("q", 8)], [("a", 7), ("a", 8)]]

                def q_emit(win, xk_r, st, items, xo=0):
                    for kind, v in items:
                        if kind == "q":
                            qproj_mi(win, xk_r, st, v, xo)
                        else:
                            qadd(win, st, v)

                # DMA order: HWDGE generates one DMA per ~625ns, so emit in
                # consumption order: wk+xk1+xk2 interleaved, tables, wv, wq;
                # bulky late-needed constants (masks, iden, wo) afterwards.
                wqk_r, xw12 = [], []
                for i, (k0, kl) in enumerate(CCQ):
                    t = wp.tile([128, 2 * C + 2 * HKV * D], bf16,
                                tag=f"wqk{i}", name=f"wqk{i}")
                    nc.sync.dma_start(t[:kl, :], wqk[k0:k0 + kl, :])
                    wqk_r.append(t)
                    k0x, klx = CCX[i]
                    tx = xkp.tile([128, 1024], bf16, tag=f"xw12_{i}",
                                  name=f"xw12_{i}")
                    nc.sync.dma_start(tx[:klx, :], xkT[k0x:k0x + klx, 512:1536])
                    xw12.append(tx)
                nc.sync.dma_start(c2k_t[:], c2k[:])
                nc.sync.dma_start(s2k_t[:], s2k[:])
                wv_r = load_w(wvp, CCX, 195, "wv")
                wq_b = (wqk_r, 0)
                wk_b = (wqk_r, 2 * C)
                xw03 = [xkp.tile([128, 1024], bf16, tag=f"xw03_{i}",
                                 name=f"xw03_{i}") for i in range(5)]
                xk1, xk2 = xw12, xw12

                # Front: K win1, K win2, Q win1 (eager adds; slot-3 scores
                # can start right after h0's add), then the V chunks slot 3
                # needs (6,7,8,9). Scores only gate on kt/qth; PV waits V a
                # little longer without stalling the exp stream.
                stk1, stk2, stq1 = {}, {}, {}
                kproj_a(1, xw12, stk1, 0)
                kproj_b(1, xw12, stk1, 0)
                kproj_a(2, xw12, stk2, 512)
                kproj_b(2, xw12, stk2, 512)
                q_emit(1, xw12, stq1, Q_SEQ[0:3], 0)
                vproj(1, xw12, 2, 0)
                vproj(1, xw12, 3, 0)
                vproj(2, xw12, 0, 512)
                vproj(2, xw12, 1, 512)
                nc.sync.dma_start(m_b[:], masksp[:])
                nc.sync.dma_start(id_t[:], idenp[:])
                for i, (k0, kl) in enumerate(MM):
                    nc.sync.dma_start(wo_r[i][:kl, :], woT[k0:k0 + kl, :])
                for items in Q_UNITS:
                    fillers.append(
                        (lambda it: lambda: q_emit(1, xw12, stq1, it, 0))
                        (items))
                fillers.append(lambda: vproj(2, xw12, 2, 512))
                fillers.append(lambda: vproj(2, xw12, 3, 512))
                fillers.append(lambda: vproj(1, xw12, 0, 0))
                fillers.append(lambda: vproj(1, xw12, 1, 0))

                def win_filler(win, with_q):
                    st = {}
                    xo = 0 if win == 0 else 512

                    def f_load():
                        load_x03(xw03, win, xo)

                    def f_ka1():
                        kproj_a1(win, xw03, st, xo)

                    def f_ka2():
                        kproj_a2(win, xw03, st, xo)

                    def f_kb():
                        kproj_b(win, xw03, st, xo)

                    def f_v(ti):
                        return lambda: vproj(win, xw03, ti, xo)

                    def f_qs(items):
                        return lambda: q_emit(win, xw03, st, items, xo)

                    kv = [f_ka1, f_ka2, f_v(0), f_kb,
                          f_v(1), f_v(2), f_v(3)]
                    qs = ([f_qs(Q_SEQ[0:2]), f_qs(Q_SEQ[2:3])]
                          + [f_qs(it) for it in Q_UNITS]) if with_q else []
                    return {"load": f_load, "kv": kv, "q": qs}

                w0 = win_filler(0, True)
                w3 = win_filler(3, False)
                # attn3/2 carry: remaining win1 Q, deferred V, win0 load+Q
                # (Q of win0 must finish before slot 1), win3 K/V.
                # win0's K/V pops inside slot 1 (guards enforce per-group
                # readiness of kt 2,3 / v 0-3 at the diag groups).
                fillers.append(w0["load"])
                fillers.extend(w0["q"])
                attn_slot(3)
                fillers.extend([w3["load"]] + w3["kv"])
                attn_slot(2, pump_plan=[2] * 18)
                fillers.extend(w0["kv"])
                attn_slot(1)
                while fillers:
                    pump()

            # ---------------- transposes, out-proj, last slot ----------------
            with (
                tc.tile_pool(name="psT", bufs=1, space="PSUM") as psT,
                tc.tile_pool(name="psR", bufs=1, space="PSUM") as psR,
            ):
                def transp_block(qc, m):
                    def g():
                        mc0, mrows = MM[m]
                        pt = psT.tile([128, 128], bf16, tag="pt", name="pt",
                                      padded_shape=[128, 1024])
                        nc.tensor.matmul(pt[:mrows, :],
                                         yq[qc][:, mc0:mc0 + mrows],
                                         id_t[:], start=True, stop=True,
                                         is_transpose=True)
                        nc.vector.tensor_copy(
                            ypr[m][0:mrows, 128 * qc:128 * (qc + 1)],
                            pt[:mrows, :])
                    return g

                ost_cur = {}

                def oproj_m(qq, m, pool=None):
                    # qq = 256-col q-quarter; quarter 2s/2s+1 ready as soon
                    # as slot s is transposed, so most of oproj hides inside
                    # the remaining attention stream. The tail quarter
                    # ping-pongs psR with the (idle by then) psS pool. The 5
                    # m-chunk results stage into one [128, 1280] tile and
                    # leave as a single [576, 256] DMA.
                    def g():
                        mc0, mrows = MM[m]
                        po = pool if pool is not None else psR
                        tg = "scores" if po is psS else "pjr"
                        shp = [128, 1024] if po is psS else [128, 256]
                        ps = po.tile(shp, f32, tag=tg, name=tg,
                                     padded_shape=[128, 512] if po is psR
                                     else None)
                        for p, (pc0, pl) in enumerate(MM):
                            nc.tensor.matmul(
                                ps[:mrows, 0:256],
                                wo_r[p][:pl, mc0:mc0 + mrows],
                                ypr[p][:pl, 256 * qq:256 * (qq + 1)],
                                start=(p == 0), stop=(p == 4))
                        if m == 0:
                            ost_cur[qq] = ostp.tile(
                                [128, 1280], f32, tag="ostage", name="ostage")
                        ost = ost_cur[qq]
                        nc.vector.tensor_copy(
                            ost[:mrows, 256 * m:256 * m + 256],
                            ps[:mrows, 0:256])
                        if m == 4:
                            nc.sync.dma_start(
                                yT[:, :, 256 * qq:256 * (qq + 1)],
                                ost[:, :].rearrange("p (m q) -> p m q", m=5))
                    return g

                for m in range(5):
                    fillers.append(transp_block(6, m))
                    fillers.append(transp_block(7, m))
                fillers.extend([oproj_m(3, m) for m in range(5)])
                for m in range(5):
                    fillers.append(transp_block(4, m))
                    fillers.append(transp_block(5, m))
                fillers.extend([oproj_m(2, m) for m in range(5)])
                for m in range(5):
                    fillers.append(transp_block(2, m))
                    fillers.append(transp_block(3, m))
                fillers.extend([oproj_m(1, m) for m in range(5)])

                def slot0_hook(h):
                    # yq[0]/yq[1] cols for feature-chunk m complete once
                    # heads 2m and 2m+1 have divided; transpose them now so
                    # only oproj qq0 remains after the slot.
                    if h % 2 == 1 and h >= 1:
                        m = (h - 1) // 2
                        fillers.append(transp_block(0, m))
                        fillers.append(transp_block(1, m))
                    elif h == 8:
                        fillers.append(transp_block(0, 4))
                        fillers.append(transp_block(1, 4))

                attn_slot(0, after_head=slot0_hook)
                while fillers:
                    pump()
                for m in range(5):
                    oproj_m(0, m, psS if m % 2 == 1 else psR)()

    nc.compile()
    return nc


def _get_program():
    global _PROG
    if _PROG is None:
        _PROG = _build_program()
    return _PROG


def _neox_perm(nheads, swap=False):
    p = []
    for h in range(nheads):
        ev = [64 * h + 2 * j for j in range(32)]
        od = [64 * h + 2 * j + 1 for j in range(32)]
        p += (od + ev) if swap else (ev + od)
    return np.array(p)


_CONSTS = None


def _static_consts():
    """Input-independent per-core constants (tables, masks, key orders)."""
    global _CONSTS
    if _CONSTS is not None:
        return _CONSTS
    invf = THETA ** (-np.arange(32, dtype=np.float64) / 32)

    def tables(pos):
        ang = pos[None, :] * invf[:, None]
        cos, sin = np.cos(ang), np.sin(ang)
        c2 = np.tile(cos, (4, 1)).astype(np.float32)
        s2 = np.tile(np.vstack([-sin, sin]), (2, 1)).astype(np.float32)
        return c2, s2

    per_j = []
    for j in range(2):
        keypos = np.concatenate(
            [np.arange(QB * q, QB * (q + 1)) for q in KEYORDER[j]])
        qsel = keypos[:TQ]          # queries = first 1024 permuted keys
        c2k, s2k = tables(keypos.astype(np.float64))
        masks = np.zeros((16 * 128, QB), np.float32)
        for s in range(4):
            seq = _slot_seq(s)
            qpos = keypos[QB * s:QB * (s + 1)]
            for k in range(4):
                c = seq[-4 + k]
                kpos = keypos[128 * c:128 * (c + 1)]
                masks[(4 * s + k) * 128:(4 * s + k + 1) * 128] = (
                    kpos[:, None] <= qpos[None, :]).astype(np.float32)
        # device layout: [128, 16*QB] (16 chunk-masks side by side)
        masks2 = masks.reshape(16, 128, QB).transpose(1, 0, 2).reshape(128, 16 * QB)
        per_j.append((keypos, qsel,
                      c2k.astype(ml_dtypes.bfloat16),
                      s2k.astype(ml_dtypes.bfloat16),
                      masks2.astype(ml_dtypes.bfloat16)))
    _CONSTS = per_j
    return _CONSTS


def _host_prep(x, Wq, Wk, Wv, Wo):
    bf = ml_dtypes.bfloat16
    wqk = np.hstack([Wq[_neox_perm(H)].T,
                     Wq[_neox_perm(H, swap=True)].T,
                     Wk[_neox_perm(HKV)].T,
                     Wk[_neox_perm(HKV, swap=True)].T]).astype(bf)
    woT = Wo.T.astype(bf)
    wvp = np.zeros((577, 195), np.float32)
    for g in range(HKV):
        wvp[:C, 65 * g:65 * g + 64] = Wv[64 * g:64 * g + 64].T
        wvp[576, 65 * g + 64] = 1.0
    wvp = wvp.astype(bf)
    iden = np.eye(128, dtype=np.float32).astype(bf)

    per_j = _static_consts()
    ones = np.ones((1, T), np.float32)
    in_maps = []
    core_meta = []
    for b in range(B):
        xbT = x[b].T
        for j in range(2):
            keypos, qsel, c2k, s2k, masks = per_j[j]
            xkT = np.vstack([xbT[:, keypos], ones]).astype(bf)
            in_maps.append({
                "xkT": xkT,
                "wqk": wqk, "wvp": wvp, "woT": woT,
                "c2k": c2k, "s2k": s2k,
                "masks": masks, "iden": iden,
            })
            core_meta.append((b, qsel))
    return in_maps, core_meta


def kernel(x, Wq, Wk, Wv, Wo):
    x = np.asarray(x, np.float32)
    Wq = np.asarray(Wq, np.float32)
    Wk = np.asarray(Wk, np.float32)
    Wv = np.asarray(Wv, np.float32)
    Wo = np.asarray(Wo, np.float32)

    from concourse.bass_utils import run_bass_kernel_spmd

    nc = _get_program()
    in_maps, core_meta = _host_prep(x, Wq, Wk, Wv, Wo)
    res = run_bass_kernel_spmd(nc, in_maps, list(range(8)))

    out = np.empty((B, T, C), np.float32)
    for core, (b, qsel) in enumerate(core_meta):
        y2 = np.asarray(res.results[core]["yT"])      # [128, 5, TQ]
        yfull = np.moveaxis(y2, 1, 0).reshape(640, TQ)[0:C]
        out[b, qsel, :] = yfull.T
    return out
